# revision 30
# baseline (speedup 1.0000x reference)
"""DigitCapsules dynamic-routing kernel for 8 Trainium2 NeuronCores.

Problem: x [64, 2048, 8] f32, W [1, 2048, 32, 16, 8] f32 ->
  u_hat[b,i,j,o] = sum_d W[0,i,j,o,d] * x[b,i,d]
  3 routing iterations (softmax over j=32 caps, weighted sum over i=2048,
  squash over o=16, agreement update), output v [64, 32, 16].

Strategy: shard in_caps (i) across the 8 cores (256 i's each). Each core's
W-slice lives in SBUF for the whole kernel; u_hat (which would be 268MB
materialized) is recomputed on the PE per routing pass from SBUF-resident
operands, so after the initial load there is NO DRAM streaming. The only
cross-core traffic is an AllReduce of the per-core partial s_j [64,32,16]
(131KB) once per iteration. Routing state b_ij is i-sharded, fully local.

v2 performance structure (vs v1 which ran the agreement reduce as a
strided 1x-mode TENSOR_REDUCE):
  - the o-reduction sum_o u*v runs as a 4-level binary tree of contiguous
    fp16 tensor_tensor adds (2x packed DVE mode), with the last level and
    part of the c*u weighting offloaded to GpSimd
  - PSUM->SBUF u_hat copies are paired into [128,1024] ACT copies on ScalarE
  - per-bg work is software-pipelined with a one-bg lag (softmax and the
    s-reduction of bg run while bg+1's u_hat matmuls fill PSUM) so the PE
    engine queue never blocks on the DVE chain and HAM stays warm

Device layouts (per core):
  K-partitions (i16, d): k = i16*8 + d       (16 i's x 8 in_dims = 128)
  M-partitions (ip, b8): p = ip*8 + b8       (16 i's x 8 batch = 128)
  w_sb  [128, 16*512]  : [(i16,d), (it, o, j)]         -- W slice
  xs_in [128, 16*64]   : [(i16,d), (it, b)]            -- x slice (pass-A lhsT)
  bdx   [128, 16*8*128]: [(i16,d), (it, bg, ip, b8)]   -- block-diag x (lhsT)
  u_hat tile (it, bg)  = bdx_tile.T @ w_tile -> PSUM [(ip,b8), (o,j)=512]
"""
import sys

sys.path.insert(0, "/opt/trn_rl_repo")

import numpy as np
import concourse.bass as bass
import concourse.mybir as mybir
import concourse.tile as tile
from concourse.vector_clock import ScopedClock
from concourse.bass_utils import run_bass_kernel_spmd

# ---------------------------------------------------------------------------
# Workaround: this walrus build rejects semaphore waits attached to InstDrain
# ("Too many sync wait commands", CoreV3GenImpl setupSyncWait NO_STRUCT) and
# allows at most one wait per instruction. Emit bare drains + sequencer-level
# barriers, and hoist the Tile tail-drain waits onto single-wait NOPs.
# ---------------------------------------------------------------------------


def _safe_multi_engine_barrier(self, engines):
    for eng_type in engines:
        d = mybir.InstDrain(
            name=self.get_next_instruction_name(),
            ins=[],
            outs=[],
            bass_is_fusable=False,
        )
        d.engine = eng_type
        self.engines[eng_type].add_instruction(d)
    for inst in self._sem_only_all_engine_barrier_insts(f"aeb{self.next_id()}"):
        self.engines[inst.engine].add_instruction(inst)


def _safe_drain_and_barrier(self, tick_clock, wait_clock):
    nop_inst = self.nc.sync.nop(nofuse=True)
    wait_clock.add_sem_waits(nop_inst.ins, ScopedClock({None: tick_clock.global_clock}))
    waits = list(nop_inst.ins.sync_info.on_wait or [])
    if len(waits) > 1:
        si = nop_inst.ins.sync_info
        si.on_wait = waits[:1]
        nop_inst.ins.sync_info = si
        for w in waits[1:]:
            extra = self.nc.sync.nop(nofuse=True)
            extra.ins.sync_info = mybir.SyncInfo(on_wait=[w], on_update=[])
    self.nc.sync.drain()
    self.nc.all_engine_barrier()
    assert self.sems is not None
    popped = self.nc._tile_sem_poison_stack.pop()
    assert popped is self._sem_poison
    self.nc.clear_and_free_semaphores(list(self.sems.allocated().values()))
    self.nc.all_engine_barrier()


bass.Bass.multi_engine_barrier = _safe_multi_engine_barrier
tile.TileContext._drain_and_barrier = _safe_drain_and_barrier


def _split_multi_waits(nc):
    """This walrus encodes at most ONE semaphore wait per instruction (zero
    on InstDrain). Hoist excess waits onto single-wait NOPs inserted just
    before the instruction on the same engine — identical semantics, since
    each engine executes its block subsequence in order."""
    uid = 0
    for f in nc.m.functions:
        for blk in f.blocks:
            out = []
            changed = False
            for inst in blk.instructions:
                si = getattr(inst, "sync_info", None)
                waits = list(si.on_wait) if si is not None and si.on_wait else []
                limit = 0 if isinstance(inst, mybir.InstDrain) else 1
                if len(waits) > limit:
                    for w in waits[: len(waits) - limit]:
                        nop = mybir.InstNoOp(
                            name=f"{inst.name}-wsplit{uid}", ins=[], outs=[])
                        uid += 1
                        nop.engine = inst.engine
                        nop.sync_info = mybir.SyncInfo(on_wait=[w], on_update=[])
                        out.append(nop)
                    inst.sync_info = mybir.SyncInfo(
                        on_wait=waits[len(waits) - limit:],
                        on_update=list(si.on_update or []),
                    )
                    changed = True
                out.append(inst)
            if changed:
                blk.instructions = out

# ---------------------------------------------------------------------------
# Problem constants (hardcoded per contract)
# ---------------------------------------------------------------------------
B, I, J, O, D = 64, 2048, 32, 16, 8
N_CORES = 8
IL = I // N_CORES          # 256 local in_caps per core
IT = IL // 16              # 16 i-tiles of 16 i's
NBG = B // 8               # 8 batch groups of 8
JO = J * O                 # 512
EPS = 1e-8
F32 = mybir.dt.float32
F16 = mybir.dt.float16
AX = mybir.AxisListType
ALU = mybir.AluOpType
ACTF = mybir.ActivationFunctionType


def build_nc(detect_races=True):
    nc = bass.Bass(num_devices=N_CORES, detect_race_conditions=detect_races)
    w_in = nc.dram_tensor("w_in", [128, IT * JO], F16, kind="ExternalInput")
    xs_in = nc.dram_tensor("xs_in", [128, IT * B], F16, kind="ExternalInput")
    bdx_in = nc.dram_tensor("bdx_in", [128, IT * NBG * 128], F16, kind="ExternalInput")
    ones_in = nc.dram_tensor("ones_in", [128, 8], F16, kind="ExternalInput")
    rep_in = nc.dram_tensor("rep_in", [32, 4 * 128], F16, kind="ExternalInput")
    v_out = nc.dram_tensor("v_out", [B, JO], F32, kind="ExternalOutput")

    groups = [list(range(N_CORES))]

    with tile.TileContext(nc) as tc:
        with (
            tc.tile_pool(name="res", bufs=1) as res,
            tc.tile_pool(name="u16p", bufs=3) as u16p,
            tc.tile_pool(name="uvp", bufs=1) as uvp,
            tc.tile_pool(name="t1p", bufs=1) as t1p,
            tc.tile_pool(name="t2p", bufs=1) as t2p,
            tc.tile_pool(name="t3p", bufs=2) as t3p,
            tc.tile_pool(name="cup", bufs=3) as cup,
            tc.tile_pool(name="smp", bufs=2) as smp,
            tc.tile_pool(name="spartp", bufs=1) as spartp,
            tc.tile_pool(name="small", bufs=4) as small,
            tc.tile_pool(name="sq", bufs=2) as sqp,
            tc.tile_pool(name="upsum", bufs=2, space="PSUM") as upsum,
            tc.tile_pool(name="spsum", bufs=1, space="PSUM") as spsum,
            tc.tile_pool(name="dram", bufs=2, space="DRAM") as dram,
        ):
            # ---- resident tiles ----
            w_sb = res.tile([128, IT * JO], F16)
            xs_sb = res.tile([128, IT * B], F16)
            bdx_sb = res.tile([128, IT * NBG * 128], F16)
            ones_sb = res.tile([128, 8], F16)
            rep_sb = res.tile([32, 4 * 128], F16)
            # fp16 routing logits: values are O(1e-2)
            b_state = res.tile([128, NBG * IT * J], F16)
            vrep = res.tile([128, NBG * JO], F16)
            eps_sb = res.tile([B, 1], F32)
            nc.gpsimd.memset(eps_sb[:], EPS)

            # pass A's critical path needs xs+w first; bdx is only needed by
            # pass B's u-builds, so it loads last.
            nc.sync.dma_start(out=xs_sb[:], in_=xs_in[:])
            for q in range(4):
                qs = (IT * JO) // 4
                nc.sync.dma_start(out=w_sb[:, q * qs:(q + 1) * qs],
                                  in_=w_in[:, q * qs:(q + 1) * qs])
            nc.sync.dma_start(out=ones_sb[:], in_=ones_in[:])
            nc.sync.dma_start(out=rep_sb[:], in_=rep_in[:])
            for q in range(4):
                qs = (IT * NBG * 128) // 4
                nc.sync.dma_start(out=bdx_sb[:, q * qs:(q + 1) * qs],
                                  in_=bdx_in[:, q * qs:(q + 1) * qs])

            def allreduce_rows(spart_sb, bg0, nbg):
                """AllReduce bgs [bg0, bg0+nbg) -> s [nbg*8, 512].

                Splitting the collective into row groups pipelines the ~20us
                per-collective latency against compute. spart_sb is [64, 512]
                (pass A, rows (bg,b8)) or [8, NBG*512] with cols (bg, o, j)
                (passes B/C)."""
                rows = nbg * 8
                part = dram.tile([rows, JO], F32, tag=f"part_{nbg}")
                ar = dram.tile([rows, JO], F32, tag=f"ar_{nbg}")
                if spart_sb.shape[0] == B:
                    nc.sync.dma_start(
                        out=part[:],
                        in_=spart_sb[bg0 * 8:bg0 * 8 + rows, :])
                else:
                    # part[bg*8+b', jo] = spart_sb[b', (bg0+bg)*512+jo]
                    # (keep the SBUF partition dim outermost in the AP)
                    src = spart_sb[:, bg0 * JO:(bg0 + nbg) * JO].rearrange(
                        "b (bg f) -> b bg f", f=JO)
                    dst = part[:].rearrange("(bg b) f -> b bg f", b=8)
                    nc.sync.dma_start(out=dst, in_=src)
                nc.gpsimd.collective_compute(
                    "AllReduce", ALU.add, replica_groups=groups,
                    ins=[part.opt()], outs=[ar.opt()],
                )
                s_r = sqp.tile([rows, JO], F32, tag=f"s_{nbg}")
                nc.sync.dma_start(out=s_r[:], in_=ar[:])
                return s_r

            def squash(s_sb):
                """v = s * s2/((1+s2)*sqrt(s2+eps)) over o; s_sb [rows,512]."""
                rows = s_sb.shape[0]
                s3 = s_sb[:].rearrange("p (o j) -> p o j", j=J)
                sq = sqp.tile([rows, JO], F32, tag=f"sq_{rows}")
                nc.vector.tensor_mul(sq[:], s_sb[:], s_sb[:])
                s2 = small.tile([rows, J], F32, tag=f"sq_s2_{rows}")
                nc.vector.tensor_reduce(
                    s2[:], sq[:].rearrange("p (o j) -> p j o", j=J), AX.X, ALU.add)
                rt = small.tile([rows, J], F32, tag=f"sq_rt_{rows}")
                nc.scalar.activation(rt[:], s2[:], ACTF.Sqrt,
                                     bias=eps_sb[0:rows, :])
                opl = small.tile([rows, J], F32, tag=f"sq_op_{rows}")
                nc.vector.tensor_scalar_add(opl[:], s2[:], 1.0)
                den = small.tile([rows, J], F32, tag=f"sq_den_{rows}")
                nc.vector.tensor_mul(den[:], opl[:], rt[:])
                rec = small.tile([rows, J], F32, tag=f"sq_rec_{rows}")
                nc.vector.reciprocal(rec[:], den[:])
                f = small.tile([rows, J], F32, tag=f"sq_f_{rows}")
                nc.vector.tensor_mul(f[:], s2[:], rec[:])
                v_sb = sqp.tile([rows, JO], F32, tag=f"v_sb_{rows}")
                nc.vector.tensor_tensor(
                    v_sb[:].rearrange("p (o j) -> p o j", j=J),
                    s3,
                    f[:].unsqueeze(1).broadcast_to([rows, O, J]),
                    op=ALU.mult,
                )
                return v_sb

            def build_vrep(v_r, bg0, nbg):
                # Replicate v rows across the 16 i-groups with one selection
                # matmul per bg: vrep[(ip,b8), bg-cols] = v[bg*8+b8, :] via
                # lhsT slice bgl of rep_sb [32, 4*128] with
                # rep[(bgl',b8), bgl*128+m] = (bgl'==bgl and m%8==b8).
                # v_r [nbg*8, 512] covers bgs bg0..bg0+nbg.
                rows = nbg * 8
                v16 = sqp.tile([rows, JO], F16, tag=f"v16_{nbg}")
                nc.vector.tensor_copy(v16[:], v_r[:])
                for bgl in range(nbg):
                    cols = slice((bg0 + bgl) * JO, (bg0 + bgl + 1) * JO)
                    vr = spsum.tile([128, JO], F32, tag="vr")
                    nc.tensor.matmul(
                        vr[:], lhsT=rep_sb[0:rows, bgl * 128:(bgl + 1) * 128],
                        rhs=v16[:], start=True, stop=True)
                    nc.scalar.copy(vrep[:, cols], vr[:])

            # ---- pass A: s0 = (1/32) * sum_i u_hat ----
            s0p = spsum.tile([B, JO], F32, tag="s0p")
            for it in range(IT):
                nc.tensor.matmul(
                    s0p[:],
                    lhsT=xs_sb[:, it * B:(it + 1) * B],
                    rhs=w_sb[:, it * JO:(it + 1) * JO],
                    start=(it == 0), stop=(it == IT - 1),
                )
            spart_a = sqp.tile([B, JO], F32, tag="spart")
            nc.scalar.mul(spart_a[:], s0p[:], 1.0 / J)
            # Quarter-split: the first collective pays the bring-up cost;
            # quarters pipeline so bg0's vrep (and pass B's first agreement)
            # starts as early as possible.
            for q in range(4):
                s_q = allreduce_rows(spart_a, q * 2, 2)
                build_vrep(squash(s_q), q * 2, 2)

            # ---- per-bg building blocks for passes B/C ----

            def u_build(bg, sfn):
                """16 matmuls (it) -> PSUM pairs -> one [128,1024] ACT copy
                each to the fp16 u tile [(ip,b8), (it,o,j)]. The deferred
                s-reduction matmuls of bg-2 (sfn callbacks) are woven between
                pairs so the PE engine queue never runs dry and HAM stays
                warm."""
                u16_bg = u16p.tile([128, IT * JO], F16, tag="u16")
                for pair in range(IT // 2):
                    up = upsum.tile([128, 2 * JO], F32)
                    for k in range(2):
                        it = pair * 2 + k
                        nc.tensor.matmul(
                            up[:, k * JO:(k + 1) * JO],
                            lhsT=bdx_sb[:, (it * NBG + bg) * 128:(it * NBG + bg + 1) * 128],
                            rhs=w_sb[:, it * JO:(it + 1) * JO],
                            start=True, stop=True,
                        )
                    nc.scalar.copy(
                        u16_bg[:, pair * 2 * JO:(pair + 1) * 2 * JO], up[:])
                    if sfn is not None and pair % 2 == 1:
                        sfn(pair // 2)
                return u16_bg

            def agreement(bg, u16_bg, first):
                """b[bg] (+)= sum_o u*v via 2x-mode fp16 tree adds.

                Per half (8 i-tiles): uv = u16 * vrep (bcast over it), then a
                4-level binary tree over o: 16 -> 8 -> 4 -> 2 -> 1. Levels
                1-3 on DVE (contiguous step-1 slices), level 4 + the b-state
                update on GpSimd."""
                vslice = vrep[:, bg * JO:(bg + 1) * JO]
                bslice = b_state[:, bg * IT * J:(bg + 1) * IT * J]
                for h in range(2):
                    hc = slice(h * 8 * JO, (h + 1) * 8 * JO)
                    uvh = uvp.tile([128, 8 * JO], F16, tag="uv")
                    # two 2048-elem multiplies: measured faster than one
                    # 4096-elem op (per-broadcast-row bubble scales with size)
                    for g in range(2):
                        gc = slice(g * 4 * JO, (g + 1) * 4 * JO)
                        nc.vector.tensor_tensor(
                            uvh[:, gc].rearrange("p (t f) -> p t f", f=JO),
                            u16_bg[:, hc][:, gc].rearrange(
                                "p (t f) -> p t f", f=JO),
                            vslice.unsqueeze(1).broadcast_to([128, 4, JO]),
                            op=ALU.mult,
                        )
                    t1 = t1p.tile([128, 8 * 8 * J], F16, tag="t1")
                    uv3 = uvh[:].rearrange("p (t f) -> p t f", f=JO)
                    nc.vector.tensor_tensor(
                        t1[:].rearrange("p (t f) -> p t f", f=8 * J),
                        uv3[:, :, 0:8 * J], uv3[:, :, 8 * J:16 * J],
                        op=ALU.add,
                    )
                    t2 = t2p.tile([128, 8 * 4 * J], F16, tag="t2")
                    t13 = t1[:].rearrange("p (t f) -> p t f", f=8 * J)
                    nc.vector.tensor_tensor(
                        t2[:].rearrange("p (t f) -> p t f", f=4 * J),
                        t13[:, :, 0:4 * J], t13[:, :, 4 * J:8 * J],
                        op=ALU.add,
                    )
                    t3 = t3p.tile([128, 8 * 2 * J], F16, tag="t3")
                    t23 = t2[:].rearrange("p (t f) -> p t f", f=4 * J)
                    nc.vector.tensor_tensor(
                        t3[:].rearrange("p (t f) -> p t f", f=2 * J),
                        t23[:, :, 0:2 * J], t23[:, :, 2 * J:4 * J],
                        op=ALU.add,
                    )
                    bsh = bslice[:, h * 8 * J:(h + 1) * 8 * J]
                    t33 = t3[:].rearrange("p (t f) -> p t f", f=2 * J)
                    if first:
                        nc.gpsimd.tensor_tensor(
                            bsh.rearrange("p (t j) -> p t j", j=J),
                            t33[:, :, 0:J], t33[:, :, J:2 * J],
                            op=ALU.add,
                        )
                    else:
                        t4 = t3p.tile([128, 8 * J], F16, tag="t4")
                        nc.gpsimd.tensor_tensor(
                            t4[:].rearrange("p (t j) -> p t j", j=J),
                            t33[:, :, 0:J], t33[:, :, J:2 * J],
                            op=ALU.add,
                        )
                        nc.gpsimd.tensor_add(bsh, bsh, t4[:])

            def softmax(bg):
                """softmax_j(b[bg]) split multiplicatively: returns ex =
                exp(b) and ebd, the 1/sum_j ex normalizer laid out as the
                block-diagonal lhsT for the s-reduction matmuls (so the
                normalize-multiply over all 512 u-columns never happens —
                it rides the contraction for free). Logits are O(1e-2) so
                exp without max-subtraction is safe."""
                bslice = b_state[:, bg * IT * J:(bg + 1) * IT * J]
                ex = smp.tile([128, IT * J], F16, tag="ex")
                nc.scalar.activation(ex[:], bslice, ACTF.Exp)
                esum = smp.tile([128, IT], F32, tag="esum")
                nc.vector.tensor_reduce(
                    esum[:], ex[:].rearrange("p (t j) -> p t j", j=J),
                    AX.X, ALU.add)
                erec = smp.tile([128, IT], F32, tag="erec")
                nc.vector.reciprocal(erec[:], esum[:])
                # ebd[p, (t,e)] = erec[p,t] * (e == p%8): mask-multiply with
                # the ones block-diagonal constant
                ebd = smp.tile([128, IT * 8], F16, tag="ebd")
                nc.vector.tensor_tensor(
                    ebd[:].rearrange("p (t e) -> p t e", e=8),
                    erec[:].unsqueeze(2).broadcast_to([128, IT, 8]),
                    ones_sb[:].unsqueeze(1).broadcast_to([128, IT, 8]),
                    op=ALU.mult,
                )
                return ex, ebd

            def s_cu(bg, ex, u16_bg):
                """cu = ex*u for all 4 quads of bg (3 DVE + 1 GpSimd).
                Returns the cu tiles for the deferred s-matmuls."""
                cus = []
                for q in range(4):
                    cuq = cup.tile([128, 4 * JO], F16, tag="cu")
                    eng = nc.gpsimd if q == 3 else nc.vector
                    eng.tensor_tensor(
                        cuq[:].rearrange("p (t o j) -> p t o j", o=O, j=J),
                        u16_bg[:, q * 4 * JO:(q + 1) * 4 * JO]
                        .rearrange("p (t o j) -> p t o j", o=O, j=J),
                        ex[:, q * 4 * J:(q + 1) * 4 * J]
                        .rearrange("p (t j) -> p t j", j=J)
                        .unsqueeze(2).broadcast_to([128, 4, O, J]),
                        op=ALU.mult,
                    )
                    cus.append(cuq)
                return cus

            def make_sfn(bg, cus, ebd, spart):
                """Callback emitting quad q's PSUM-accumulated s-matmuls
                (sum over the 16 ip's, weighted by the softmax normalizer in
                the block-diag lhsT); woven into bg+2's u_build."""
                sp = spsum.tile([8, JO], F32, tag="sp")

                def sfn(q):
                    for k in range(4):
                        it = q * 4 + k
                        nc.tensor.matmul(
                            sp[:], lhsT=ebd[:, it * 8:(it + 1) * 8],
                            rhs=cus[q][:, k * JO:(k + 1) * JO],
                            start=(it == 0), stop=(it == IT - 1))
                    if q == 3:
                        nc.scalar.copy(spart[:, bg * JO:(bg + 1) * JO], sp[:])
                return sfn

            # ---- passes B (iter 1) and C (iter 2) ----
            # Two-bg software pipeline: iteration bg emits softmax(bg-2) and
            # its cu quads, then u_build(bg) with bg-2's ones-matmuls woven
            # between the u-matmul pairs, then agreement(bg). Every engine's
            # FIFO then only ever waits on work that is already in flight,
            # and the PE never idles long enough for HAM to re-throttle.
            for pass_idx in (1, 2):
                first = pass_idx == 1
                spart = spartp.tile([8, NBG * JO], F32, tag="spart_bc")
                pend = []
                s_h0 = None
                for bg in range(NBG + 2):
                    sfn = None
                    if bg >= 2:
                        pbg, pu16 = pend.pop(0)
                        ex, ebd = softmax(pbg)
                        cus = s_cu(pbg, ex, pu16)
                        sfn = make_sfn(pbg, cus, ebd, spart)
                    if bg < NBG:
                        u16_bg = u_build(bg, sfn)
                        agreement(bg, u16_bg, first)
                        pend.append((bg, u16_bg))
                    elif sfn is not None:
                        for q in range(4):
                            sfn(q)
                    if bg >= 2 and pbg == 3:
                        # half 0's collective overlaps bgs 4-7's compute
                        s_h0 = allreduce_rows(spart, 0, 4)
                    if bg >= 2 and pbg == 5 and pass_idx == 2:
                        # output pass: finer AR split shortens the tail
                        s_q2 = allreduce_rows(spart, 4, 2)
                if pass_idx == 1:
                    s_h1 = allreduce_rows(spart, 4, 4)
                    build_vrep(squash(s_h0), 0, 4)
                    build_vrep(squash(s_h1), 4, 4)
                else:
                    s_q3 = allreduce_rows(spart, 6, 2)
                    for s_r, bg0, nbg in ((s_h0, 0, 4), (s_q2, 4, 2),
                                          (s_q3, 6, 2)):
                        v_r = squash(s_r)
                        nc.sync.dma_start(
                            out=v_out[bg0 * 8:(bg0 + nbg) * 8, :], in_=v_r[:])
    _split_multi_waits(nc)
    return nc


def prep_inputs(x, W):
    """Host-side layout prep. x [64,2048,8] f32, W [1,2048,32,16,8] f32."""
    x = np.ascontiguousarray(x, dtype=np.float32).astype(np.float16)
    Wf = np.ascontiguousarray(W, dtype=np.float32)[0].astype(np.float16)
    in_maps = []
    ones_bd = np.tile(np.eye(8, dtype=np.float16), (16, 1))  # [(i16,b8), 8]
    # rep[(bgl,b8), (bgl', (ip,b8'))] = (bgl'==bgl and b8'==b8)
    rep = np.zeros((4, 8, 4, 16, 8), dtype=np.float16)
    for bgl in range(4):
        for b8 in range(8):
            rep[bgl, b8, bgl, :, b8] = 1.0
    rep = rep.reshape(32, 512)
    for c in range(N_CORES):
        i0 = c * IL
        Wl = Wf[i0:i0 + IL].reshape(IT, 16, J, O, D)         # [it, i16, j, o, d]
        w_in = np.ascontiguousarray(
            Wl.transpose(1, 4, 0, 3, 2)).reshape(128, IT * JO)  # (i16,d),(it,o,j)
        xl = x[:, i0:i0 + IL, :].reshape(B, IT, 16, D)        # [b, it, i16, d]
        xt = np.ascontiguousarray(xl.transpose(2, 3, 1, 0))   # [i16, d, it, b]
        xs_in = xt.reshape(128, IT * B)
        # block-diag x: [i16, d, it, bg, ip, b8], nonzero at ip == i16
        bdx = np.zeros((16, D, IT, NBG, 16, 8), dtype=np.float16)
        xg = xt.reshape(16, D, IT, NBG, 8)                    # [i16, d, it, bg, b8]
        idx = np.arange(16)
        bdx[idx, :, :, :, idx, :] = xg[idx]
        in_maps.append({
            "w_in": w_in,
            "xs_in": xs_in,
            "bdx_in": bdx.reshape(128, IT * NBG * 128),
            "ones_in": ones_bd,
            "rep_in": rep,
        })
    return in_maps


def postprocess(v_raw):
    """Device v_out is [B, (o,j)]; return [B, J, O]."""
    return np.ascontiguousarray(
        np.asarray(v_raw).reshape(B, O, J).transpose(0, 2, 1))


def kernel(x, W):
    nc = build_nc()
    in_maps = prep_inputs(np.asarray(x), np.asarray(W))
    res = run_bass_kernel_spmd(nc, in_maps, core_ids=list(range(N_CORES)))
    return postprocess(res.results[0]["v_out"])


if __name__ == "__main__":
    rng = np.random.default_rng(0)
    x = rng.standard_normal((B, I, D), dtype=np.float32)
    W = (0.01 * rng.standard_normal((1, I, J, O, D))).astype(np.float32)
    v = kernel(x, W)
    print("kernel output", v.shape, v.dtype, float(np.abs(v).max()))


# revision 31
# speedup vs baseline: 1.2735x; 1.2735x over previous
"""DigitCapsules dynamic-routing kernel for 8 Trainium2 NeuronCores.

Problem: x [64, 2048, 8] f32, W [1, 2048, 32, 16, 8] f32 ->
  u_hat[b,i,j,o] = sum_d W[0,i,j,o,d] * x[b,i,d]
  3 routing iterations (softmax over j=32 caps, weighted sum over i=2048,
  squash over o=16, agreement update), output v [64, 32, 16].

Strategy: shard in_caps (i) across the 8 cores (256 i's each). Each core's
W-slice lives in SBUF for the whole kernel; u_hat (which would be 268MB
materialized) is recomputed on the PE per routing pass from SBUF-resident
operands, so after the initial load there is NO DRAM streaming. The only
cross-core traffic is an AllReduce of the per-core partial s_j [64,32,16]
(131KB) once per iteration. Routing state b_ij is i-sharded, fully local.

v2 performance structure (vs v1 which ran the agreement reduce as a
strided 1x-mode TENSOR_REDUCE):
  - the o-reduction sum_o u*v runs as a 4-level binary tree of contiguous
    fp16 tensor_tensor adds (2x packed DVE mode), with the last level and
    part of the c*u weighting offloaded to GpSimd
  - PSUM->SBUF u_hat copies are paired into [128,1024] ACT copies on ScalarE
  - per-bg work is software-pipelined with a one-bg lag (softmax and the
    s-reduction of bg run while bg+1's u_hat matmuls fill PSUM) so the PE
    engine queue never blocks on the DVE chain and HAM stays warm

Device layouts (per core):
  K-partitions (i16, d): k = i16*8 + d       (16 i's x 8 in_dims = 128)
  M-partitions (ip, b8): p = ip*8 + b8       (16 i's x 8 batch = 128)
  w_sb  [128, 16*512]  : [(i16,d), (it, o, j)]         -- W slice
  xs_in [128, 16*64]   : [(i16,d), (it, b)]            -- x slice (pass-A lhsT)
  bdx   [128, 16*8*128]: [(i16,d), (it, bg, ip, b8)]   -- block-diag x (lhsT)
  u_hat tile (it, bg)  = bdx_tile.T @ w_tile -> PSUM [(ip,b8), (o,j)=512]
"""
import sys

sys.path.insert(0, "/opt/trn_rl_repo")

import numpy as np
import concourse.bass as bass
import concourse.mybir as mybir
import concourse.tile as tile
from concourse.vector_clock import ScopedClock
from concourse.bass_utils import run_bass_kernel_spmd

# ---------------------------------------------------------------------------
# Workaround: this walrus build rejects semaphore waits attached to InstDrain
# ("Too many sync wait commands", CoreV3GenImpl setupSyncWait NO_STRUCT) and
# allows at most one wait per instruction. Emit bare drains + sequencer-level
# barriers, and hoist the Tile tail-drain waits onto single-wait NOPs.
# ---------------------------------------------------------------------------


def _safe_multi_engine_barrier(self, engines):
    for eng_type in engines:
        d = mybir.InstDrain(
            name=self.get_next_instruction_name(),
            ins=[],
            outs=[],
            bass_is_fusable=False,
        )
        d.engine = eng_type
        self.engines[eng_type].add_instruction(d)
    for inst in self._sem_only_all_engine_barrier_insts(f"aeb{self.next_id()}"):
        self.engines[inst.engine].add_instruction(inst)


def _safe_drain_and_barrier(self, tick_clock, wait_clock):
    nop_inst = self.nc.sync.nop(nofuse=True)
    wait_clock.add_sem_waits(nop_inst.ins, ScopedClock({None: tick_clock.global_clock}))
    waits = list(nop_inst.ins.sync_info.on_wait or [])
    if len(waits) > 1:
        si = nop_inst.ins.sync_info
        si.on_wait = waits[:1]
        nop_inst.ins.sync_info = si
        for w in waits[1:]:
            extra = self.nc.sync.nop(nofuse=True)
            extra.ins.sync_info = mybir.SyncInfo(on_wait=[w], on_update=[])
    self.nc.sync.drain()
    self.nc.all_engine_barrier()
    assert self.sems is not None
    popped = self.nc._tile_sem_poison_stack.pop()
    assert popped is self._sem_poison
    self.nc.clear_and_free_semaphores(list(self.sems.allocated().values()))
    self.nc.all_engine_barrier()


bass.Bass.multi_engine_barrier = _safe_multi_engine_barrier
tile.TileContext._drain_and_barrier = _safe_drain_and_barrier


def _split_multi_waits(nc):
    """This walrus encodes at most ONE semaphore wait per instruction (zero
    on InstDrain). Hoist excess waits onto single-wait NOPs inserted just
    before the instruction on the same engine — identical semantics, since
    each engine executes its block subsequence in order."""
    uid = 0
    for f in nc.m.functions:
        for blk in f.blocks:
            out = []
            changed = False
            for inst in blk.instructions:
                si = getattr(inst, "sync_info", None)
                waits = list(si.on_wait) if si is not None and si.on_wait else []
                limit = 0 if isinstance(inst, mybir.InstDrain) else 1
                if len(waits) > limit:
                    for w in waits[: len(waits) - limit]:
                        nop = mybir.InstNoOp(
                            name=f"{inst.name}-wsplit{uid}", ins=[], outs=[])
                        uid += 1
                        nop.engine = inst.engine
                        nop.sync_info = mybir.SyncInfo(on_wait=[w], on_update=[])
                        out.append(nop)
                    inst.sync_info = mybir.SyncInfo(
                        on_wait=waits[len(waits) - limit:],
                        on_update=list(si.on_update or []),
                    )
                    changed = True
                out.append(inst)
            if changed:
                blk.instructions = out

# ---------------------------------------------------------------------------
# Problem constants (hardcoded per contract)
# ---------------------------------------------------------------------------
B, I, J, O, D = 64, 2048, 32, 16, 8
N_CORES = 8
IL = I // N_CORES          # 256 local in_caps per core
IT = IL // 16              # 16 i-tiles of 16 i's
NBG = B // 8               # 8 batch groups of 8
JO = J * O                 # 512
EPS = 1e-8
F32 = mybir.dt.float32
F16 = mybir.dt.float16
AX = mybir.AxisListType
ALU = mybir.AluOpType
ACTF = mybir.ActivationFunctionType


def build_nc(detect_races=True):
    nc = bass.Bass(num_devices=N_CORES, detect_race_conditions=detect_races)
    w_in = nc.dram_tensor("w_in", [128, IT * JO], F16, kind="ExternalInput")
    xs_in = nc.dram_tensor("xs_in", [128, IT * B], F16, kind="ExternalInput")
    bdx_in = nc.dram_tensor("bdx_in", [128, IT * NBG * 128], F16, kind="ExternalInput")
    ones_in = nc.dram_tensor("ones_in", [128, 8], F16, kind="ExternalInput")
    rep_in = nc.dram_tensor("rep_in", [32, 4 * 128], F16, kind="ExternalInput")
    v_out = nc.dram_tensor("v_out", [B, JO], F32, kind="ExternalOutput")

    groups = [list(range(N_CORES))]

    with tile.TileContext(nc) as tc:
        with (
            tc.tile_pool(name="res", bufs=1) as res,
            tc.tile_pool(name="u16p", bufs=3) as u16p,
            tc.tile_pool(name="uvp", bufs=1) as uvp,
            tc.tile_pool(name="t1p", bufs=1) as t1p,
            tc.tile_pool(name="t2p", bufs=1) as t2p,
            tc.tile_pool(name="t3p", bufs=2) as t3p,
            tc.tile_pool(name="cup", bufs=3) as cup,
            tc.tile_pool(name="smp", bufs=2) as smp,
            tc.tile_pool(name="spartp", bufs=1) as spartp,
            tc.tile_pool(name="small", bufs=4) as small,
            tc.tile_pool(name="sq", bufs=2) as sqp,
            tc.tile_pool(name="upsum", bufs=2, space="PSUM") as upsum,
            tc.tile_pool(name="spsum", bufs=1, space="PSUM") as spsum,
            tc.tile_pool(name="dram", bufs=2, space="DRAM") as dram,
        ):
            # ---- resident tiles ----
            w_sb = res.tile([128, IT * JO], F16)
            xs_sb = res.tile([128, IT * B], F16)
            bdx_sb = res.tile([128, IT * NBG * 128], F16)
            ones_sb = res.tile([128, 8], F16)
            rep_sb = res.tile([32, 4 * 128], F16)
            # fp16 routing logits: values are O(1e-2)
            b_state = res.tile([128, NBG * IT * J], F16)
            vrep = res.tile([128, NBG * JO], F16)
            eps_sb = res.tile([B, 1], F32)
            nc.gpsimd.memset(eps_sb[:], EPS)

            # pass A's critical path needs xs+w first; bdx is only needed by
            # pass B's u-builds, so it loads last.
            nc.sync.dma_start(out=xs_sb[:], in_=xs_in[:])
            for q in range(4):
                qs = (IT * JO) // 4
                nc.sync.dma_start(out=w_sb[:, q * qs:(q + 1) * qs],
                                  in_=w_in[:, q * qs:(q + 1) * qs])
            nc.sync.dma_start(out=ones_sb[:], in_=ones_in[:])
            nc.sync.dma_start(out=rep_sb[:], in_=rep_in[:])
            for q in range(4):
                qs = (IT * NBG * 128) // 4
                nc.sync.dma_start(out=bdx_sb[:, q * qs:(q + 1) * qs],
                                  in_=bdx_in[:, q * qs:(q + 1) * qs])

            def allreduce_rows(spart_sb, bg0, nbg):
                """AllReduce bgs [bg0, bg0+nbg) -> s [nbg*8, 512].

                Splitting the collective into row groups pipelines the ~20us
                per-collective latency against compute. spart_sb is [64, 512]
                (pass A, rows (bg,b8)) or [8, NBG*512] with cols (bg, o, j)
                (passes B/C)."""
                rows = nbg * 8
                part = dram.tile([rows, JO], F32, tag=f"part_{nbg}")
                ar = dram.tile([rows, JO], F32, tag=f"ar_{nbg}")
                if spart_sb.shape[0] == B:
                    nc.sync.dma_start(
                        out=part[:],
                        in_=spart_sb[bg0 * 8:bg0 * 8 + rows, :])
                else:
                    # part[bg*8+b', jo] = spart_sb[b', (bg0+bg)*512+jo]
                    # (keep the SBUF partition dim outermost in the AP)
                    src = spart_sb[:, bg0 * JO:(bg0 + nbg) * JO].rearrange(
                        "b (bg f) -> b bg f", f=JO)
                    dst = part[:].rearrange("(bg b) f -> b bg f", b=8)
                    nc.sync.dma_start(out=dst, in_=src)
                nc.gpsimd.collective_compute(
                    "AllReduce", ALU.add, replica_groups=groups,
                    ins=[part.opt()], outs=[ar.opt()],
                )
                s_r = sqp.tile([rows, JO], F32, tag=f"s_{nbg}")
                nc.sync.dma_start(out=s_r[:], in_=ar[:])
                return s_r

            def squash(s_sb):
                """v = s * s2/((1+s2)*sqrt(s2+eps)) over o; s_sb [rows,512]."""
                rows = s_sb.shape[0]
                s3 = s_sb[:].rearrange("p (o j) -> p o j", j=J)
                sq = sqp.tile([rows, JO], F32, tag=f"sq_{rows}")
                nc.vector.tensor_mul(sq[:], s_sb[:], s_sb[:])
                s2 = small.tile([rows, J], F32, tag=f"sq_s2_{rows}")
                nc.vector.tensor_reduce(
                    s2[:], sq[:].rearrange("p (o j) -> p j o", j=J), AX.X, ALU.add)
                rt = small.tile([rows, J], F32, tag=f"sq_rt_{rows}")
                nc.scalar.activation(rt[:], s2[:], ACTF.Sqrt,
                                     bias=eps_sb[0:rows, :])
                opl = small.tile([rows, J], F32, tag=f"sq_op_{rows}")
                nc.vector.tensor_scalar_add(opl[:], s2[:], 1.0)
                den = small.tile([rows, J], F32, tag=f"sq_den_{rows}")
                nc.vector.tensor_mul(den[:], opl[:], rt[:])
                rec = small.tile([rows, J], F32, tag=f"sq_rec_{rows}")
                nc.vector.reciprocal(rec[:], den[:])
                f = small.tile([rows, J], F32, tag=f"sq_f_{rows}")
                nc.vector.tensor_mul(f[:], s2[:], rec[:])
                v_sb = sqp.tile([rows, JO], F32, tag=f"v_sb_{rows}")
                nc.vector.tensor_tensor(
                    v_sb[:].rearrange("p (o j) -> p o j", j=J),
                    s3,
                    f[:].unsqueeze(1).broadcast_to([rows, O, J]),
                    op=ALU.mult,
                )
                return v_sb

            def build_vrep(v_r, bg0, nbg):
                # Replicate v rows across the 16 i-groups with one selection
                # matmul per bg: vrep[(ip,b8), bg-cols] = v[bg*8+b8, :] via
                # lhsT slice bgl of rep_sb [32, 4*128] with
                # rep[(bgl',b8), bgl*128+m] = (bgl'==bgl and m%8==b8).
                # v_r [nbg*8, 512] covers bgs bg0..bg0+nbg.
                rows = nbg * 8
                v16 = sqp.tile([rows, JO], F16, tag=f"v16_{nbg}")
                nc.vector.tensor_copy(v16[:], v_r[:])
                for bgl in range(nbg):
                    cols = slice((bg0 + bgl) * JO, (bg0 + bgl + 1) * JO)
                    vr = spsum.tile([128, JO], F32, tag="vr")
                    nc.tensor.matmul(
                        vr[:], lhsT=rep_sb[0:rows, bgl * 128:(bgl + 1) * 128],
                        rhs=v16[:], start=True, stop=True)
                    nc.scalar.copy(vrep[:, cols], vr[:])

            # ---- pass A: s0 = (1/32) * sum_i u_hat ----
            s0p = spsum.tile([B, JO], F32, tag="s0p")
            for it in range(IT):
                nc.tensor.matmul(
                    s0p[:],
                    lhsT=xs_sb[:, it * B:(it + 1) * B],
                    rhs=w_sb[:, it * JO:(it + 1) * JO],
                    start=(it == 0), stop=(it == IT - 1),
                )
            spart_a = sqp.tile([B, JO], F32, tag="spart")
            nc.scalar.mul(spart_a[:], s0p[:], 1.0 / J)
            # Quarter-split: the first collective pays the bring-up cost;
            # quarters pipeline so bg0's vrep (and pass B's first agreement)
            # starts as early as possible.
            for q in range(4):
                s_q = allreduce_rows(spart_a, q * 2, 2)
                build_vrep(squash(s_q), q * 2, 2)

            # ---- per-bg building blocks for passes B/C ----

            def u_build(bg, sfn):
                """16 matmuls (it) -> PSUM pairs -> one [128,1024] ACT copy
                each to the fp16 u tile [(ip,b8), (it,o,j)]. The deferred
                s-reduction matmuls of bg-2 (sfn callbacks) are woven between
                pairs so the PE engine queue never runs dry and HAM stays
                warm."""
                u16_bg = u16p.tile([128, IT * JO], F16, tag="u16")
                for pair in range(IT // 2):
                    up = upsum.tile([128, 2 * JO], F32)
                    for k in range(2):
                        it = pair * 2 + k
                        nc.tensor.matmul(
                            up[:, k * JO:(k + 1) * JO],
                            lhsT=bdx_sb[:, (it * NBG + bg) * 128:(it * NBG + bg + 1) * 128],
                            rhs=w_sb[:, it * JO:(it + 1) * JO],
                            start=True, stop=True,
                        )
                    nc.scalar.copy(
                        u16_bg[:, pair * 2 * JO:(pair + 1) * 2 * JO], up[:])
                    if sfn is not None and pair % 2 == 1:
                        sfn(pair // 2)
                return u16_bg

            def agreement(bg, u16_bg, first):
                """b[bg] (+)= sum_o u*v via 2x-mode fp16 tree adds.

                Per half (8 i-tiles): uv = u16 * vrep (bcast over it), then a
                4-level binary tree over o: 16 -> 8 -> 4 -> 2 -> 1. Levels
                1-3 on DVE (contiguous step-1 slices), level 4 + the b-state
                update on GpSimd."""
                vslice = vrep[:, bg * JO:(bg + 1) * JO]
                bslice = b_state[:, bg * IT * J:(bg + 1) * IT * J]
                for h in range(2):
                    hc = slice(h * 8 * JO, (h + 1) * 8 * JO)
                    uvh = uvp.tile([128, 8 * JO], F16, tag="uv")
                    # two 2048-elem multiplies: measured faster than one
                    # 4096-elem op (per-broadcast-row bubble scales with size)
                    for g in range(2):
                        gc = slice(g * 4 * JO, (g + 1) * 4 * JO)
                        nc.vector.tensor_tensor(
                            uvh[:, gc].rearrange("p (t f) -> p t f", f=JO),
                            u16_bg[:, hc][:, gc].rearrange(
                                "p (t f) -> p t f", f=JO),
                            vslice.unsqueeze(1).broadcast_to([128, 4, JO]),
                            op=ALU.mult,
                        )
                    t1 = t1p.tile([128, 8 * 8 * J], F16, tag="t1")
                    uv3 = uvh[:].rearrange("p (t f) -> p t f", f=JO)
                    nc.vector.tensor_tensor(
                        t1[:].rearrange("p (t f) -> p t f", f=8 * J),
                        uv3[:, :, 0:8 * J], uv3[:, :, 8 * J:16 * J],
                        op=ALU.add,
                    )
                    t2 = t2p.tile([128, 8 * 4 * J], F16, tag="t2")
                    t13 = t1[:].rearrange("p (t f) -> p t f", f=8 * J)
                    nc.vector.tensor_tensor(
                        t2[:].rearrange("p (t f) -> p t f", f=4 * J),
                        t13[:, :, 0:4 * J], t13[:, :, 4 * J:8 * J],
                        op=ALU.add,
                    )
                    t3 = t3p.tile([128, 8 * 2 * J], F16, tag="t3")
                    t23 = t2[:].rearrange("p (t f) -> p t f", f=4 * J)
                    nc.vector.tensor_tensor(
                        t3[:].rearrange("p (t f) -> p t f", f=2 * J),
                        t23[:, :, 0:2 * J], t23[:, :, 2 * J:4 * J],
                        op=ALU.add,
                    )
                    bsh = bslice[:, h * 8 * J:(h + 1) * 8 * J]
                    t33 = t3[:].rearrange("p (t f) -> p t f", f=2 * J)
                    if first:
                        nc.gpsimd.tensor_tensor(
                            bsh.rearrange("p (t j) -> p t j", j=J),
                            t33[:, :, 0:J], t33[:, :, J:2 * J],
                            op=ALU.add,
                        )
                    else:
                        t4 = t3p.tile([128, 8 * J], F16, tag="t4")
                        nc.gpsimd.tensor_tensor(
                            t4[:].rearrange("p (t j) -> p t j", j=J),
                            t33[:, :, 0:J], t33[:, :, J:2 * J],
                            op=ALU.add,
                        )
                        nc.gpsimd.tensor_add(bsh, bsh, t4[:])

            def softmax(bg):
                """softmax_j(b[bg]) split multiplicatively: returns ex =
                exp(b) and ebd, the 1/sum_j ex normalizer laid out as the
                block-diagonal lhsT for the s-reduction matmuls (so the
                normalize-multiply over all 512 u-columns never happens —
                it rides the contraction for free). Logits are O(1e-2) so
                exp without max-subtraction is safe."""
                bslice = b_state[:, bg * IT * J:(bg + 1) * IT * J]
                ex = smp.tile([128, IT * J], F16, tag="ex")
                nc.scalar.activation(ex[:], bslice, ACTF.Exp)
                esum = smp.tile([128, IT], F32, tag="esum")
                nc.vector.tensor_reduce(
                    esum[:], ex[:].rearrange("p (t j) -> p t j", j=J),
                    AX.X, ALU.add)
                erec = smp.tile([128, IT], F32, tag="erec")
                nc.vector.reciprocal(erec[:], esum[:])
                # ebd[p, (t,e)] = erec[p,t] * (e == p%8): mask-multiply with
                # the ones block-diagonal constant
                ebd = smp.tile([128, IT * 8], F16, tag="ebd")
                nc.vector.tensor_tensor(
                    ebd[:].rearrange("p (t e) -> p t e", e=8),
                    erec[:].unsqueeze(2).broadcast_to([128, IT, 8]),
                    ones_sb[:].unsqueeze(1).broadcast_to([128, IT, 8]),
                    op=ALU.mult,
                )
                return ex, ebd

            def s_cu(bg, ex, u16_bg):
                """cu = ex*u for all 4 quads of bg (DVE; GpSimd co-streaming
                here taxes every concurrent DVE op ~20% via SBUF contention).
                Returns the cu tiles for the deferred s-matmuls."""
                cus = []
                for q in range(4):
                    cuq = cup.tile([128, 4 * JO], F16, tag="cu")
                    nc.vector.tensor_tensor(
                        cuq[:].rearrange("p (t o j) -> p t o j", o=O, j=J),
                        u16_bg[:, q * 4 * JO:(q + 1) * 4 * JO]
                        .rearrange("p (t o j) -> p t o j", o=O, j=J),
                        ex[:, q * 4 * J:(q + 1) * 4 * J]
                        .rearrange("p (t j) -> p t j", j=J)
                        .unsqueeze(2).broadcast_to([128, 4, O, J]),
                        op=ALU.mult,
                    )
                    cus.append(cuq)
                return cus

            def make_sfn(bg, cus, ebd, spart):
                """Callback emitting quad q's PSUM-accumulated s-matmuls
                (sum over the 16 ip's, weighted by the softmax normalizer in
                the block-diag lhsT); woven into bg+2's u_build."""
                sp = spsum.tile([8, JO], F32, tag="sp")

                def sfn(q):
                    for k in range(4):
                        it = q * 4 + k
                        nc.tensor.matmul(
                            sp[:], lhsT=ebd[:, it * 8:(it + 1) * 8],
                            rhs=cus[q][:, k * JO:(k + 1) * JO],
                            start=(it == 0), stop=(it == IT - 1))
                    if q == 3:
                        nc.scalar.copy(spart[:, bg * JO:(bg + 1) * JO], sp[:])
                return sfn

            # ---- passes B (iter 1) and C (iter 2) ----
            # Two-bg software pipeline: iteration bg emits softmax(bg-2) and
            # its cu quads, then u_build(bg) with bg-2's ones-matmuls woven
            # between the u-matmul pairs, then agreement(bg). Every engine's
            # FIFO then only ever waits on work that is already in flight,
            # and the PE never idles long enough for HAM to re-throttle.
            for pass_idx in (1, 2):
                first = pass_idx == 1
                spart = spartp.tile([8, NBG * JO], F32, tag="spart_bc")
                pend = []
                s_h0 = None
                for bg in range(NBG + 2):
                    sfn = None
                    if bg >= 2:
                        pbg, pu16 = pend.pop(0)
                        ex, ebd = softmax(pbg)
                        cus = s_cu(pbg, ex, pu16)
                        sfn = make_sfn(pbg, cus, ebd, spart)
                    if bg < NBG:
                        u16_bg = u_build(bg, sfn)
                        agreement(bg, u16_bg, first)
                        pend.append((bg, u16_bg))
                    elif sfn is not None:
                        for q in range(4):
                            sfn(q)
                    if bg >= 2 and pbg == 3:
                        # half 0's collective overlaps bgs 4-7's compute
                        s_h0 = allreduce_rows(spart, 0, 4)
                    if bg >= 2 and pbg == 5 and pass_idx == 2:
                        # output pass: finer AR split shortens the tail
                        s_q2 = allreduce_rows(spart, 4, 2)
                if pass_idx == 1:
                    s_h1 = allreduce_rows(spart, 4, 4)
                    build_vrep(squash(s_h0), 0, 4)
                    build_vrep(squash(s_h1), 4, 4)
                else:
                    s_q3 = allreduce_rows(spart, 6, 2)
                    for s_r, bg0, nbg in ((s_h0, 0, 4), (s_q2, 4, 2),
                                          (s_q3, 6, 2)):
                        v_r = squash(s_r)
                        nc.sync.dma_start(
                            out=v_out[bg0 * 8:(bg0 + nbg) * 8, :], in_=v_r[:])
    _split_multi_waits(nc)
    return nc


def prep_inputs(x, W):
    """Host-side layout prep. x [64,2048,8] f32, W [1,2048,32,16,8] f32."""
    x = np.ascontiguousarray(x, dtype=np.float32).astype(np.float16)
    Wf = np.ascontiguousarray(W, dtype=np.float32)[0].astype(np.float16)
    in_maps = []
    ones_bd = np.tile(np.eye(8, dtype=np.float16), (16, 1))  # [(i16,b8), 8]
    # rep[(bgl,b8), (bgl', (ip,b8'))] = (bgl'==bgl and b8'==b8)
    rep = np.zeros((4, 8, 4, 16, 8), dtype=np.float16)
    for bgl in range(4):
        for b8 in range(8):
            rep[bgl, b8, bgl, :, b8] = 1.0
    rep = rep.reshape(32, 512)
    for c in range(N_CORES):
        i0 = c * IL
        Wl = Wf[i0:i0 + IL].reshape(IT, 16, J, O, D)         # [it, i16, j, o, d]
        w_in = np.ascontiguousarray(
            Wl.transpose(1, 4, 0, 3, 2)).reshape(128, IT * JO)  # (i16,d),(it,o,j)
        xl = x[:, i0:i0 + IL, :].reshape(B, IT, 16, D)        # [b, it, i16, d]
        xt = np.ascontiguousarray(xl.transpose(2, 3, 1, 0))   # [i16, d, it, b]
        xs_in = xt.reshape(128, IT * B)
        # block-diag x: [i16, d, it, bg, ip, b8], nonzero at ip == i16
        bdx = np.zeros((16, D, IT, NBG, 16, 8), dtype=np.float16)
        xg = xt.reshape(16, D, IT, NBG, 8)                    # [i16, d, it, bg, b8]
        idx = np.arange(16)
        bdx[idx, :, :, :, idx, :] = xg[idx]
        in_maps.append({
            "w_in": w_in,
            "xs_in": xs_in,
            "bdx_in": bdx.reshape(128, IT * NBG * 128),
            "ones_in": ones_bd,
            "rep_in": rep,
        })
    return in_maps


def postprocess(v_raw):
    """Device v_out is [B, (o,j)]; return [B, J, O]."""
    return np.ascontiguousarray(
        np.asarray(v_raw).reshape(B, O, J).transpose(0, 2, 1))


def kernel(x, W):
    nc = build_nc()
    in_maps = prep_inputs(np.asarray(x), np.asarray(W))
    res = run_bass_kernel_spmd(nc, in_maps, core_ids=list(range(N_CORES)))
    return postprocess(res.results[0]["v_out"])


if __name__ == "__main__":
    rng = np.random.default_rng(0)
    x = rng.standard_normal((B, I, D), dtype=np.float32)
    W = (0.01 * rng.standard_normal((1, I, J, O, D))).astype(np.float32)
    v = kernel(x, W)
    print("kernel output", v.shape, v.dtype, float(np.abs(v).max()))


# revision 38
# speedup vs baseline: 1.2900x; 1.0130x over previous
"""DigitCapsules dynamic-routing kernel for 8 Trainium2 NeuronCores.

Problem: x [64, 2048, 8] f32, W [1, 2048, 32, 16, 8] f32 ->
  u_hat[b,i,j,o] = sum_d W[0,i,j,o,d] * x[b,i,d]
  3 routing iterations (softmax over j=32 caps, weighted sum over i=2048,
  squash over o=16, agreement update), output v [64, 32, 16].

Strategy: shard in_caps (i) across the 8 cores (256 i's each). Each core's
W-slice lives in SBUF for the whole kernel; u_hat (which would be 268MB
materialized) is recomputed on the PE per routing pass from SBUF-resident
operands, so after the initial load there is NO DRAM streaming. The only
cross-core traffic is an AllReduce of the per-core partial s_j [64,32,16]
(131KB) once per iteration. Routing state b_ij is i-sharded, fully local.

v2 performance structure (vs v1 which ran the agreement reduce as a
strided 1x-mode TENSOR_REDUCE):
  - the o-reduction sum_o u*v runs as a 4-level binary tree of contiguous
    fp16 tensor_tensor adds (2x packed DVE mode), with the last level and
    part of the c*u weighting offloaded to GpSimd
  - PSUM->SBUF u_hat copies are paired into [128,1024] ACT copies on ScalarE
  - per-bg work is software-pipelined with a one-bg lag (softmax and the
    s-reduction of bg run while bg+1's u_hat matmuls fill PSUM) so the PE
    engine queue never blocks on the DVE chain and HAM stays warm

Device layouts (per core):
  K-partitions (i16, d): k = i16*8 + d       (16 i's x 8 in_dims = 128)
  M-partitions (ip, b8): p = ip*8 + b8       (16 i's x 8 batch = 128)
  w_sb  [128, 16*512]  : [(i16,d), (it, o, j)]         -- W slice
  xs_in [128, 16*64]   : [(i16,d), (it, b)]            -- x slice (pass-A lhsT)
  bdx   [128, 16*8*128]: [(i16,d), (it, bg, ip, b8)]   -- block-diag x (lhsT)
  u_hat tile (it, bg)  = bdx_tile.T @ w_tile -> PSUM [(ip,b8), (o,j)=512]
"""
import sys

sys.path.insert(0, "/opt/trn_rl_repo")

import numpy as np
import concourse.bass as bass
import concourse.mybir as mybir
import concourse.tile as tile
from concourse.vector_clock import ScopedClock
from concourse.bass_utils import run_bass_kernel_spmd

# ---------------------------------------------------------------------------
# Workaround: this walrus build rejects semaphore waits attached to InstDrain
# ("Too many sync wait commands", CoreV3GenImpl setupSyncWait NO_STRUCT) and
# allows at most one wait per instruction. Emit bare drains + sequencer-level
# barriers, and hoist the Tile tail-drain waits onto single-wait NOPs.
# ---------------------------------------------------------------------------


def _safe_multi_engine_barrier(self, engines):
    for eng_type in engines:
        d = mybir.InstDrain(
            name=self.get_next_instruction_name(),
            ins=[],
            outs=[],
            bass_is_fusable=False,
        )
        d.engine = eng_type
        self.engines[eng_type].add_instruction(d)
    for inst in self._sem_only_all_engine_barrier_insts(f"aeb{self.next_id()}"):
        self.engines[inst.engine].add_instruction(inst)


def _safe_drain_and_barrier(self, tick_clock, wait_clock):
    nop_inst = self.nc.sync.nop(nofuse=True)
    wait_clock.add_sem_waits(nop_inst.ins, ScopedClock({None: tick_clock.global_clock}))
    waits = list(nop_inst.ins.sync_info.on_wait or [])
    if len(waits) > 1:
        si = nop_inst.ins.sync_info
        si.on_wait = waits[:1]
        nop_inst.ins.sync_info = si
        for w in waits[1:]:
            extra = self.nc.sync.nop(nofuse=True)
            extra.ins.sync_info = mybir.SyncInfo(on_wait=[w], on_update=[])
    self.nc.sync.drain()
    self.nc.all_engine_barrier()
    assert self.sems is not None
    popped = self.nc._tile_sem_poison_stack.pop()
    assert popped is self._sem_poison
    self.nc.clear_and_free_semaphores(list(self.sems.allocated().values()))
    self.nc.all_engine_barrier()


bass.Bass.multi_engine_barrier = _safe_multi_engine_barrier
tile.TileContext._drain_and_barrier = _safe_drain_and_barrier


def _split_multi_waits(nc):
    """This walrus encodes at most ONE semaphore wait per instruction (zero
    on InstDrain). Hoist excess waits onto single-wait NOPs inserted just
    before the instruction on the same engine — identical semantics, since
    each engine executes its block subsequence in order."""
    uid = 0
    for f in nc.m.functions:
        for blk in f.blocks:
            out = []
            changed = False
            for inst in blk.instructions:
                si = getattr(inst, "sync_info", None)
                waits = list(si.on_wait) if si is not None and si.on_wait else []
                limit = 0 if isinstance(inst, mybir.InstDrain) else 1
                if len(waits) > limit:
                    for w in waits[: len(waits) - limit]:
                        nop = mybir.InstNoOp(
                            name=f"{inst.name}-wsplit{uid}", ins=[], outs=[])
                        uid += 1
                        nop.engine = inst.engine
                        nop.sync_info = mybir.SyncInfo(on_wait=[w], on_update=[])
                        out.append(nop)
                    inst.sync_info = mybir.SyncInfo(
                        on_wait=waits[len(waits) - limit:],
                        on_update=list(si.on_update or []),
                    )
                    changed = True
                out.append(inst)
            if changed:
                blk.instructions = out

# ---------------------------------------------------------------------------
# Problem constants (hardcoded per contract)
# ---------------------------------------------------------------------------
B, I, J, O, D = 64, 2048, 32, 16, 8
N_CORES = 8
IL = I // N_CORES          # 256 local in_caps per core
IT = IL // 16              # 16 i-tiles of 16 i's
NBG = B // 8               # 8 batch groups of 8
JO = J * O                 # 512
EPS = 1e-8
F32 = mybir.dt.float32
F16 = mybir.dt.float16
AX = mybir.AxisListType
ALU = mybir.AluOpType
ACTF = mybir.ActivationFunctionType


def build_nc(detect_races=True):
    nc = bass.Bass(num_devices=N_CORES, detect_race_conditions=detect_races)
    w_in = nc.dram_tensor("w_in", [128, IT * JO], F16, kind="ExternalInput")
    xs_in = nc.dram_tensor("xs_in", [128, IT * B], F16, kind="ExternalInput")
    bdx_in = nc.dram_tensor("bdx_in", [128, IT * NBG * 128], F16, kind="ExternalInput")
    ones_in = nc.dram_tensor("ones_in", [128, 8], F16, kind="ExternalInput")
    rep_in = nc.dram_tensor("rep_in", [32, 4 * 128], F16, kind="ExternalInput")
    v_out = nc.dram_tensor("v_out", [B, JO], F32, kind="ExternalOutput")

    groups = [list(range(N_CORES))]

    with tile.TileContext(nc) as tc:
        with (
            tc.tile_pool(name="res", bufs=1) as res,
            tc.tile_pool(name="u16p", bufs=3) as u16p,
            tc.tile_pool(name="uvp", bufs=1) as uvp,
            tc.tile_pool(name="t1p", bufs=1) as t1p,
            tc.tile_pool(name="t2p", bufs=1) as t2p,
            tc.tile_pool(name="t3p", bufs=2) as t3p,
            tc.tile_pool(name="cup", bufs=3) as cup,
            tc.tile_pool(name="smp", bufs=2) as smp,
            tc.tile_pool(name="spartp", bufs=1) as spartp,
            tc.tile_pool(name="small", bufs=4) as small,
            tc.tile_pool(name="sq", bufs=2) as sqp,
            tc.tile_pool(name="upsum", bufs=2, space="PSUM") as upsum,
            tc.tile_pool(name="spsum", bufs=1, space="PSUM") as spsum,
            tc.tile_pool(name="dram", bufs=2, space="DRAM") as dram,
        ):
            # ---- resident tiles ----
            w_sb = res.tile([128, IT * JO], F16)
            xs_sb = res.tile([128, IT * B], F16)
            bdx_sb = res.tile([128, IT * NBG * 128], F16)
            ones_sb = res.tile([128, 8], F16)
            rep_sb = res.tile([32, 4 * 128], F16)
            # fp16 routing logits: values are O(1e-2)
            b_state = res.tile([128, NBG * IT * J], F16)
            vrep = res.tile([128, NBG * JO], F16)
            eps_sb = res.tile([B, 1], F32)
            nc.gpsimd.memset(eps_sb[:], EPS)

            # pass A's critical path needs xs+w first; bdx is only needed by
            # pass B's u-builds, so it loads last.
            nc.sync.dma_start(out=xs_sb[:], in_=xs_in[:])
            for q in range(4):
                qs = (IT * JO) // 4
                nc.sync.dma_start(out=w_sb[:, q * qs:(q + 1) * qs],
                                  in_=w_in[:, q * qs:(q + 1) * qs])
            nc.sync.dma_start(out=ones_sb[:], in_=ones_in[:])
            nc.sync.dma_start(out=rep_sb[:], in_=rep_in[:])
            for q in range(4):
                qs = (IT * NBG * 128) // 4
                nc.sync.dma_start(out=bdx_sb[:, q * qs:(q + 1) * qs],
                                  in_=bdx_in[:, q * qs:(q + 1) * qs])

            def allreduce_rows(spart_sb, bg0, nbg):
                """AllReduce bgs [bg0, bg0+nbg) -> s [nbg*8, 512].

                Splitting the collective into row groups pipelines the ~20us
                per-collective latency against compute. spart_sb is [64, 512]
                (pass A, rows (bg,b8)) or [8, NBG*512] with cols (bg, o, j)
                (passes B/C)."""
                rows = nbg * 8
                part = dram.tile([rows, JO], F32, tag=f"part_{nbg}")
                ar = dram.tile([rows, JO], F32, tag=f"ar_{nbg}")
                if spart_sb.shape[0] == B:
                    nc.sync.dma_start(
                        out=part[:],
                        in_=spart_sb[bg0 * 8:bg0 * 8 + rows, :])
                else:
                    # part[bg*8+b', jo] = spart_sb[b', (bg0+bg)*512+jo]
                    # (keep the SBUF partition dim outermost in the AP)
                    src = spart_sb[:, bg0 * JO:(bg0 + nbg) * JO].rearrange(
                        "b (bg f) -> b bg f", f=JO)
                    dst = part[:].rearrange("(bg b) f -> b bg f", b=8)
                    nc.sync.dma_start(out=dst, in_=src)
                nc.gpsimd.collective_compute(
                    "AllReduce", ALU.add, replica_groups=groups,
                    ins=[part.opt()], outs=[ar.opt()],
                )
                s_r32 = sqp.tile([32, JO], F32, tag="s_ar")
                nc.sync.dma_start(out=s_r32[0:rows, :], in_=ar[:])
                return s_r32[0:rows, :]

            def squash(s_sb):
                """v = s * s2/((1+s2)*sqrt(s2+eps)) over o; s_sb [rows,512]."""
                rows = s_sb.shape[0]
                s3 = s_sb.rearrange("p (o j) -> p o j", j=J)
                sq = sqp.tile([32, JO], F32, tag="sq", name="sq")[0:rows, :]
                nc.vector.tensor_mul(sq, s_sb, s_sb)
                s2 = small.tile([32, J], F32, tag="sq_s2", name="sq_s2")[0:rows, :]
                nc.vector.tensor_reduce(
                    s2, sq.rearrange("p (o j) -> p j o", j=J), AX.X, ALU.add)
                rt = small.tile([32, J], F32, tag="sq_rt", name="sq_rt")[0:rows, :]
                nc.scalar.activation(rt, s2, ACTF.Sqrt,
                                     bias=eps_sb[0:rows, :])
                opl = small.tile([32, J], F32, tag="sq_op", name="sq_op")[0:rows, :]
                nc.vector.tensor_scalar_add(opl, s2, 1.0)
                den = small.tile([32, J], F32, tag="sq_den", name="sq_den")[0:rows, :]
                nc.vector.tensor_mul(den, opl, rt)
                rec = small.tile([32, J], F32, tag="sq_rec", name="sq_rec")[0:rows, :]
                nc.vector.reciprocal(rec, den)
                f = small.tile([32, J], F32, tag="sq_f", name="sq_f")[0:rows, :]
                nc.vector.tensor_mul(f, s2, rec)
                v_sb = sqp.tile([32, JO], F32, tag="v_sb", name="v_sb")[0:rows, :]
                nc.vector.tensor_tensor(
                    v_sb.rearrange("p (o j) -> p o j", j=J),
                    s3,
                    f.unsqueeze(1).broadcast_to([rows, O, J]),
                    op=ALU.mult,
                )
                return v_sb

            def build_vrep(v_r, bg0, nbg):
                # Replicate v rows across the 16 i-groups with one selection
                # matmul per bg: vrep[(ip,b8), bg-cols] = v[bg*8+b8, :] via
                # lhsT slice bgl of rep_sb [32, 4*128] with
                # rep[(bgl',b8), bgl*128+m] = (bgl'==bgl and m%8==b8).
                # v_r [nbg*8, 512] covers bgs bg0..bg0+nbg.
                rows = nbg * 8
                v16 = sqp.tile([32, JO], F16, tag="v16", name="v16")[0:rows, :]
                nc.vector.tensor_copy(v16, v_r)
                for bgl in range(nbg):
                    cols = slice((bg0 + bgl) * JO, (bg0 + bgl + 1) * JO)
                    vr = spsum.tile([128, JO], F32, tag="vr")
                    nc.tensor.matmul(
                        vr[:], lhsT=rep_sb[0:rows, bgl * 128:(bgl + 1) * 128],
                        rhs=v16, start=True, stop=True)
                    nc.scalar.copy(vrep[:, cols], vr[:])

            # ---- pass A: s0 = (1/32) * sum_i u_hat ----
            s0p = spsum.tile([B, JO], F32, tag="s0p")
            for it in range(IT):
                nc.tensor.matmul(
                    s0p[:],
                    lhsT=xs_sb[:, it * B:(it + 1) * B],
                    rhs=w_sb[:, it * JO:(it + 1) * JO],
                    start=(it == 0), stop=(it == IT - 1),
                )
            spart_a = sqp.tile([B, JO], F32, tag="spart")
            nc.scalar.mul(spart_a[:], s0p[:], 1.0 / J)
            # Fine split: the first collective pays the bring-up cost and
            # pass B's first agreement needs only bg0's vrep, so bg0 gets
            # its own tiny AR; later groups pipeline behind it.
            for bg0, nbg in ((0, 1), (1, 1), (2, 2), (4, 2), (6, 2)):
                s_q = allreduce_rows(spart_a, bg0, nbg)
                build_vrep(squash(s_q), bg0, nbg)

            # ---- per-bg building blocks for passes B/C ----

            def u_build(bg, sfn):
                """16 matmuls (it) -> PSUM pairs -> one [128,1024] ACT copy
                each to the fp16 u tile [(ip,b8), (it,o,j)]. The deferred
                s-reduction matmuls of bg-2 (sfn callbacks) are woven between
                pairs so the PE engine queue never runs dry and HAM stays
                warm."""
                u16_bg = u16p.tile([128, IT * JO], F16, tag="u16")
                for pair in range(IT // 2):
                    up = upsum.tile([128, 2 * JO], F32)
                    for k in range(2):
                        it = pair * 2 + k
                        nc.tensor.matmul(
                            up[:, k * JO:(k + 1) * JO],
                            lhsT=bdx_sb[:, (it * NBG + bg) * 128:(it * NBG + bg + 1) * 128],
                            rhs=w_sb[:, it * JO:(it + 1) * JO],
                            start=True, stop=True,
                        )
                    nc.scalar.copy(
                        u16_bg[:, pair * 2 * JO:(pair + 1) * 2 * JO], up[:])
                    if sfn is not None and pair % 2 == 1:
                        sfn(pair // 2)
                return u16_bg

            def agreement(bg, u16_bg, first):
                """b[bg] (+)= sum_o u*v via 2x-mode fp16 tree adds.

                Per half (8 i-tiles): uv = u16 * vrep (bcast over it), then a
                4-level binary tree over o: 16 -> 8 -> 4 -> 2 -> 1. Levels
                1-3 on DVE (contiguous step-1 slices), level 4 + the b-state
                update on GpSimd."""
                vslice = vrep[:, bg * JO:(bg + 1) * JO]
                bslice = b_state[:, bg * IT * J:(bg + 1) * IT * J]
                uv = uvp.tile([128, IT * JO], F16, tag="uv")
                # four 2048-elem multiplies: measured faster than fewer
                # bigger ops (per-broadcast-row bubble scales with size)
                for g in range(4):
                    gc = slice(g * 4 * JO, (g + 1) * 4 * JO)
                    nc.vector.tensor_tensor(
                        uv[:, gc].rearrange("p (t f) -> p t f", f=JO),
                        u16_bg[:, gc].rearrange("p (t f) -> p t f", f=JO),
                        vslice.unsqueeze(1).broadcast_to([128, 4, JO]),
                        op=ALU.mult,
                    )
                # tree levels run once per bg (not per half): fewer DVE
                # instruction overheads
                t1 = t1p.tile([128, IT * 8 * J], F16, tag="t1")
                uv3 = uv[:].rearrange("p (t f) -> p t f", f=JO)
                nc.vector.tensor_tensor(
                    t1[:].rearrange("p (t f) -> p t f", f=8 * J),
                    uv3[:, :, 0:8 * J], uv3[:, :, 8 * J:16 * J],
                    op=ALU.add,
                )
                t2 = t2p.tile([128, IT * 4 * J], F16, tag="t2")
                t13 = t1[:].rearrange("p (t f) -> p t f", f=8 * J)
                nc.vector.tensor_tensor(
                    t2[:].rearrange("p (t f) -> p t f", f=4 * J),
                    t13[:, :, 0:4 * J], t13[:, :, 4 * J:8 * J],
                    op=ALU.add,
                )
                t3 = t3p.tile([128, IT * 2 * J], F16, tag="t3")
                t23 = t2[:].rearrange("p (t f) -> p t f", f=4 * J)
                nc.vector.tensor_tensor(
                    t3[:].rearrange("p (t f) -> p t f", f=2 * J),
                    t23[:, :, 0:2 * J], t23[:, :, 2 * J:4 * J],
                    op=ALU.add,
                )
                t33 = t3[:].rearrange("p (t f) -> p t f", f=2 * J)
                if first:
                    nc.gpsimd.tensor_tensor(
                        bslice.rearrange("p (t j) -> p t j", j=J),
                        t33[:, :, 0:J], t33[:, :, J:2 * J],
                        op=ALU.add,
                    )
                else:
                    t4 = t3p.tile([128, IT * J], F16, tag="t4")
                    nc.gpsimd.tensor_tensor(
                        t4[:].rearrange("p (t j) -> p t j", j=J),
                        t33[:, :, 0:J], t33[:, :, J:2 * J],
                        op=ALU.add,
                    )
                    nc.gpsimd.tensor_add(bslice, bslice, t4[:])

            def softmax(bg):
                """softmax_j(b[bg]) split multiplicatively: returns ex =
                exp(b) and ebd, the 1/sum_j ex normalizer laid out as the
                block-diagonal lhsT for the s-reduction matmuls (so the
                normalize-multiply over all 512 u-columns never happens —
                it rides the contraction for free). Logits are O(1e-2) so
                exp without max-subtraction is safe."""
                bslice = b_state[:, bg * IT * J:(bg + 1) * IT * J]
                ex = smp.tile([128, IT * J], F16, tag="ex")
                nc.scalar.activation(ex[:], bslice, ACTF.Exp)
                esum = smp.tile([128, IT], F32, tag="esum")
                nc.vector.tensor_reduce(
                    esum[:], ex[:].rearrange("p (t j) -> p t j", j=J),
                    AX.X, ALU.add)
                erec = smp.tile([128, IT], F32, tag="erec")
                nc.vector.reciprocal(erec[:], esum[:])
                # ebd[p, (t,e)] = erec[p,t] * (e == p%8): mask-multiply with
                # the ones block-diagonal constant
                ebd = smp.tile([128, IT * 8], F16, tag="ebd")
                nc.vector.tensor_tensor(
                    ebd[:].rearrange("p (t e) -> p t e", e=8),
                    erec[:].unsqueeze(2).broadcast_to([128, IT, 8]),
                    ones_sb[:].unsqueeze(1).broadcast_to([128, IT, 8]),
                    op=ALU.mult,
                )
                return ex, ebd

            def s_cu(bg, ex, u16_bg):
                """cu = ex*u for all 4 quads of bg (DVE; GpSimd co-streaming
                here taxes every concurrent DVE op ~20% via SBUF contention).
                Returns the cu tiles for the deferred s-matmuls."""
                cus = []
                for q in range(4):
                    cuq = cup.tile([128, 4 * JO], F16, tag="cu")
                    nc.vector.tensor_tensor(
                        cuq[:].rearrange("p (t o j) -> p t o j", o=O, j=J),
                        u16_bg[:, q * 4 * JO:(q + 1) * 4 * JO]
                        .rearrange("p (t o j) -> p t o j", o=O, j=J),
                        ex[:, q * 4 * J:(q + 1) * 4 * J]
                        .rearrange("p (t j) -> p t j", j=J)
                        .unsqueeze(2).broadcast_to([128, 4, O, J]),
                        op=ALU.mult,
                    )
                    cus.append(cuq)
                return cus

            def make_sfn(bg, cus, ebd, spart):
                """Callback emitting quad q's PSUM-accumulated s-matmuls
                (sum over the 16 ip's, weighted by the softmax normalizer in
                the block-diag lhsT); woven into bg+2's u_build."""
                sp = spsum.tile([8, JO], F32, tag="sp")

                def sfn(q):
                    for k in range(4):
                        it = q * 4 + k
                        nc.tensor.matmul(
                            sp[:], lhsT=ebd[:, it * 8:(it + 1) * 8],
                            rhs=cus[q][:, k * JO:(k + 1) * JO],
                            start=(it == 0), stop=(it == IT - 1))
                    if q == 3:
                        nc.scalar.copy(spart[:, bg * JO:(bg + 1) * JO], sp[:])
                return sfn

            # ---- passes B (iter 1) and C (iter 2) ----
            # Two-bg software pipeline: iteration bg emits softmax(bg-2) and
            # its cu quads, then u_build(bg) with bg-2's ones-matmuls woven
            # between the u-matmul pairs, then agreement(bg). Every engine's
            # FIFO then only ever waits on work that is already in flight,
            # and the PE never idles long enough for HAM to re-throttle.
            for pass_idx in (1, 2):
                first = pass_idx == 1
                spart = spartp.tile([8, NBG * JO], F32, tag="spart_bc")
                pend = []
                s_h0 = None
                for bg in range(NBG + 2):
                    sfn = None
                    if bg >= 2:
                        pbg, pu16 = pend.pop(0)
                        ex, ebd = softmax(pbg)
                        cus = s_cu(pbg, ex, pu16)
                        sfn = make_sfn(pbg, cus, ebd, spart)
                    if bg < NBG:
                        u16_bg = u_build(bg, sfn)
                        agreement(bg, u16_bg, first)
                        pend.append((bg, u16_bg))
                    elif sfn is not None:
                        for q in range(4):
                            sfn(q)
                    if bg >= 2 and pbg == 3:
                        # half 0's collective overlaps bgs 4-7's compute
                        s_h0 = allreduce_rows(spart, 0, 4)
                    if bg >= 2 and pbg == 5 and pass_idx == 2:
                        # output pass: finer AR split shortens the tail
                        s_q2 = allreduce_rows(spart, 4, 2)
                if pass_idx == 1:
                    s_h1 = allreduce_rows(spart, 4, 4)
                    build_vrep(squash(s_h0), 0, 4)
                    build_vrep(squash(s_h1), 4, 4)
                else:
                    s_q3 = allreduce_rows(spart, 6, 2)
                    for s_r, bg0, nbg in ((s_h0, 0, 4), (s_q2, 4, 2),
                                          (s_q3, 6, 2)):
                        v_r = squash(s_r)
                        nc.sync.dma_start(
                            out=v_out[bg0 * 8:(bg0 + nbg) * 8, :], in_=v_r)
    _split_multi_waits(nc)
    return nc


def prep_inputs(x, W):
    """Host-side layout prep. x [64,2048,8] f32, W [1,2048,32,16,8] f32."""
    x = np.ascontiguousarray(x, dtype=np.float32).astype(np.float16)
    Wf = np.ascontiguousarray(W, dtype=np.float32)[0].astype(np.float16)
    in_maps = []
    ones_bd = np.tile(np.eye(8, dtype=np.float16), (16, 1))  # [(i16,b8), 8]
    # rep[(bgl,b8), (bgl', (ip,b8'))] = (bgl'==bgl and b8'==b8)
    rep = np.zeros((4, 8, 4, 16, 8), dtype=np.float16)
    for bgl in range(4):
        for b8 in range(8):
            rep[bgl, b8, bgl, :, b8] = 1.0
    rep = rep.reshape(32, 512)
    for c in range(N_CORES):
        i0 = c * IL
        Wl = Wf[i0:i0 + IL].reshape(IT, 16, J, O, D)         # [it, i16, j, o, d]
        w_in = np.ascontiguousarray(
            Wl.transpose(1, 4, 0, 3, 2)).reshape(128, IT * JO)  # (i16,d),(it,o,j)
        xl = x[:, i0:i0 + IL, :].reshape(B, IT, 16, D)        # [b, it, i16, d]
        xt = np.ascontiguousarray(xl.transpose(2, 3, 1, 0))   # [i16, d, it, b]
        xs_in = xt.reshape(128, IT * B)
        # block-diag x: [i16, d, it, bg, ip, b8], nonzero at ip == i16
        bdx = np.zeros((16, D, IT, NBG, 16, 8), dtype=np.float16)
        xg = xt.reshape(16, D, IT, NBG, 8)                    # [i16, d, it, bg, b8]
        idx = np.arange(16)
        bdx[idx, :, :, :, idx, :] = xg[idx]
        in_maps.append({
            "w_in": w_in,
            "xs_in": xs_in,
            "bdx_in": bdx.reshape(128, IT * NBG * 128),
            "ones_in": ones_bd,
            "rep_in": rep,
        })
    return in_maps


def postprocess(v_raw):
    """Device v_out is [B, (o,j)]; return [B, J, O]."""
    return np.ascontiguousarray(
        np.asarray(v_raw).reshape(B, O, J).transpose(0, 2, 1))


def kernel(x, W):
    nc = build_nc()
    in_maps = prep_inputs(np.asarray(x), np.asarray(W))
    res = run_bass_kernel_spmd(nc, in_maps, core_ids=list(range(N_CORES)))
    return postprocess(res.results[0]["v_out"])


if __name__ == "__main__":
    rng = np.random.default_rng(0)
    x = rng.standard_normal((B, I, D), dtype=np.float32)
    W = (0.01 * rng.standard_normal((1, I, J, O, D))).astype(np.float32)
    v = kernel(x, W)
    print("kernel output", v.shape, v.dtype, float(np.abs(v).max()))
